# revision 32
# baseline (speedup 1.0000x reference)
"""Trainium2 Bass kernel for sparse_attention nn_A2_42752104464871.

Reference computation (per batch sample b):
    xr = x[b] reshaped (C=512, N=4096)
    A = wA @ xr; B = wB @ xr; V = wV @ xr          (INTER=128 each)
    A_attn = softmax(A, axis=N); V_attn = softmax(V, axis=inter)
    G = B @ A_attn^T ; Y = G @ V_attn ; Z = wP @ Y
    out = x + BN(Z) * gamma + beta                 (BN over batch+spatial)

Distribution: pure data-parallel, 2 samples per core on 8 NeuronCores;
weights replicated.  The only cross-core traffic is a 4 KB AllReduce of
the per-channel Z sums and squared sums (the wP projection of the
Y-space second moment is applied locally, before the reduce).

Implementation notes:
  - the kernel is tensor-engine-bound in phase 1 (the PE runs under a
    power throttle at ~half nominal clock), so everything movable is
    kept off the PE: weights arrive pre-transposed/pre-cast from the
    host (layout marshalling in kernel()), and the gpsimd software DMA
    queue carries only the casting x flood.
  - the attention chain runs in bf16 with fp32 PSUM accumulate.
  - softmaxes skip max-subtraction (inputs are ~N(0,1); exp is safe).
  - row-sums of expA / column-sums of expV come free as persistent
    ones-columns appended to the G and Y^T matmul moving operands.
  - AB projections, exp(A) and the B^T drain run on PAIRED spatial
    chunks (full 2KB PSUM banks); the Y^T loop runs paired too, with a
    two-column reciprocal and a broadcast tensor_tensor scale per pair.
  - BN stats: M = sum_n [Y|1]^T [Y|1] on PSUM -> local projection into
    one packed PSUM bank -> single-op strided drains -> 4KB f32
    AllReduce (Shared output).  A dummy warmup AllReduce issued during
    the x flood absorbs the collective first-call latency.
  - all activation tables pre-warm during the DMA flood.
  - sample transposes and both samples' Z matmuls run under the
    AllReduce; output is written bf16 with the affine split between
    scalar (activation) and vector (tensor_scalar), residuals on
    vector, and full-row stores alternating sync/scalar HW queues.
"""

import numpy as np
import ml_dtypes

from concourse import bacc, mybir, tile
from concourse.bass_utils import run_bass_kernel_spmd

N_CORES = 8
B_GLOBAL = 16
B_LOCAL = B_GLOBAL // N_CORES  # 2
C = 512
CC = C // 128  # 4 chunks of channels
H = W = 64
N = H * W  # 4096
NK = N // 128  # 32 spatial chunks
HALF = N // 2  # 2048
INTER = 128
EPS = 1e-5
BN_COUNT = float(B_GLOBAL * N)  # 65536
WO = 130  # chunk width incl. the two ones-columns
ROT = 4  # rotating slots for expa/bto chunk buffers (must be even)

F32 = mybir.dt.float32
BF16 = mybir.dt.bfloat16
AF = mybir.ActivationFunctionType
ALU = mybir.AluOpType


def _body(nc, cp, xp, expv_p, y_p, sp, ob_p, zb_p,
          ps_ab, ps_w, ps_y, ps_s, dp, ext):
    (x_ext, wabt_ext, wvt_ext, wptb_ext, wpn_ext, gbt_ext, idb_ext,
     out_ext) = ext

    # ---------------- preformatted weights on the scalar HW queue ---------
    # identity first: the PE warmup needs it earliest
    ident_b = cp.tile([128, 128], BF16, name="ident_b")
    wabt_all = cp.tile([128, CC * 256], BF16, name="wabt_all")
    wvt_all = cp.tile([128, CC * 128], BF16, name="wvt_all")
    wpt_b = cp.tile([128, C], BF16, name="wpt_b")
    wpn_all = cp.tile([128, C], F32, name="wpn_all")
    gb_sb = cp.tile([128, 2 * CC], F32, name="gb_sb")
    nc.scalar.dma_start(ident_b[:], idb_ext.ap())
    nc.scalar.dma_start(wabt_all[:], wabt_ext.ap())
    nc.scalar.dma_start(wvt_all[:], wvt_ext.ap())
    nc.scalar.dma_start(wpt_b[:], wptb_ext.ap())
    nc.scalar.dma_start(wpn_all[:], wpn_ext.ap())
    nc.scalar.dma_start(gb_sb[:], gbt_ext.ap())
    wabt = [wabt_all[:, c * 256:(c + 1) * 256] for c in range(CC)]
    wvt = [wvt_all[:, c * 128:(c + 1) * 128] for c in range(CC)]
    gamma_sb = gb_sb[:, 0:CC]
    beta_sb = gb_sb[:, CC:2 * CC]

    # warmup AllReduce triggered BEFORE the flood: the input is an
    # inline (NEFF-embedded) DRAM constant so the gpsimd doorbell has
    # NOTHING to wait on -- it fires ~10.5us in on every core with
    # minimal skew, completes under the x flood, and pays the collective
    # first-call latency well clear of the real AllReduce.  (A doorbell
    # that waits mid-queue stalls the software-DGE x transfers too.)
    eps_t = cp.tile([128, 1], F32, name="eps_t")
    nc.vector.memset(eps_t[:], EPS)
    war_in = nc.inline_tensor(np.zeros((128, 1), dtype=np.float32),
                              name="war_in")
    war_out = dp.tile([128, 1], F32, name="war_out", addr_space="Shared")
    nc.gpsimd.collective_compute(
        "AllReduce", ALU.add,
        replica_groups=[list(range(N_CORES))],
        ins=[war_in.ap()], outs=[war_out.opt()])

    # x flood (gpsimd: only it can cast f32->bf16).  Sample 0 half 0 lands
    # at quarter granularity so the first AB pair starts ~5us in; the rest
    # uses half-granularity DMAs (fewer gpsimd dispatch stalls).
    x_res = []
    for s in range(B_LOCAL):
        xhs = []
        for h in range(2):
            xh = [xp.tile([128, HALF], BF16, name=f"x{s}h{h}c{c}", tag="x")
                  for c in range(CC)]
            if s == 0:
                for q in range(4):
                    for c in range(CC):
                        off = h * HALF + q * 512
                        nc.gpsimd.dma_start(
                            xh[c][:, q * 512:(q + 1) * 512],
                            x_ext.ap()[s, c * 128:(c + 1) * 128,
                                       off:off + 512])
            else:
                for q in range(2):
                    for c in range(CC):
                        off = h * HALF + q * 1024
                        nc.gpsimd.dma_start(
                            xh[c][:, q * 1024:(q + 1) * 1024],
                            x_ext.ap()[s, c * 128:(c + 1) * 128,
                                       off:off + 1024])
            xhs.append(xh)
        x_res.append(xhs)

    # rotating spatial-chunk buffers; persistent yt (all 32 chunks live).
    # only the ones-COLUMNS of yt need presetting (cols 128:130 of each
    # 130-wide chunk) -- a strided memset, not a 1MB fill.
    expa_rot = cp.tile([128, ROT * 128], BF16, name="expa_rot")
    bto_rot = cp.tile([128, ROT * WO], BF16, name="bto_rot")
    yt_bufs = [cp.tile([128, NK * WO], BF16, name=f"yt_big{s}")
               for s in range(B_LOCAL)]
    for r in range(ROT):
        nc.vector.memset(bto_rot[:, r * WO + 128:r * WO + WO], 1.0)
    for t in yt_bufs:
        nc.vector.memset(
            t[:].rearrange("p (nk w) -> p nk w", nk=NK)[:, :, 128:130], 1.0)
    gt = cp.tile([128, WO], BF16, name="gt")
    nc.vector.memset(gt[:, 128:130], 1.0)

    # pre-warm every activation table used later so no ~1.3us
    # ACT_TABLE_LOAD lands on the critical path (Exp first: needed ~5us in)
    warm = sp.tile([128, 1], F32, name="warm", tag="small")
    nc.scalar.activation(warm[:], eps_t[:], AF.Identity)
    nc.scalar.activation(warm[:], eps_t[:], AF.Square)
    nc.scalar.activation(warm[:], eps_t[:], AF.Sqrt)
    nc.scalar.activation(warm[:], eps_t[:], AF.Exp)
    warm_v = sp.tile([128, 1], F32, name="warm_v", tag="small")
    nc.vector.reciprocal(warm_v[:], eps_t[:])

    # short PE warmup while the x DMAs stream
    pwu = ps_w.tile([128, 128], F32, name="pwu", tag="ps_w")
    for i in range(4):
        nc.tensor.matmul(pwu[:], ident_b[:], ident_b[:],
                         start=(i == 0), stop=(i == 3))
    wu_sink = sp.tile([128, 1], F32, name="wu_sink", tag="small")
    nc.vector.tensor_copy(wu_sink[:], pwu[:, 0:1])

    macc = cp.tile([128, WO], F32, name="macc")

    def expa_c(nk):
        r = nk % ROT
        return expa_rot[:, r * 128:(r + 1) * 128]

    def bto_c(nk):
        r = nk % ROT
        return bto_rot[:, r * WO:r * WO + WO]

    def yt_c(s, nk, full=True):
        return yt_bufs[s][:, nk * WO:nk * WO + (WO if full else 128)]

    def pair3(ap2d, width, inner):
        """view a [128, 2*width] slice as [128, 2, inner] (inner<=width)"""
        return ap2d.rearrange("p (two w) -> p two w", two=2)[:, :, 0:inner]

    # ---------------- phase 1: per-sample ----------------
    ys = []
    for s in range(B_LOCAL):
        expv = expv_p.tile([128, N], BF16, name=f"expv{s}", tag="expv")
        pg = ps_s.tile([128, WO], F32, name=f"pg{s}", tag="ps_s")

        def g_mm(j):
            nc.tensor.matmul(pg[:], expa_c(j), bto_c(j),
                             start=(j == 0), stop=(j == NK - 1))
        for h in range(2):
            xh = x_res[s][h]
            # A^T/B^T joint projection over this half's 8 chunk-PAIRS
            for kp in range(8):
                nk = h * 16 + kp * 2  # first chunk of the pair (even)
                pab = ps_ab.tile([128, 512], F32, name=f"pab{s}_{nk}",
                                 tag="ps_ab")
                for u in range(2):
                    k = kp * 2 + u
                    for c in range(CC):
                        nc.tensor.matmul(
                            pab[:, u * 256:(u + 1) * 256],
                            xh[c][:, k * 128:(k + 1) * 128], wabt[c],
                            start=(c == 0), stop=(c == CC - 1))
                # paired drains: exp(A) on scalar, B^T cast-copy on vector
                r = nk % ROT
                nc.scalar.activation(
                    expa_rot[:, r * 128:(r + 2) * 128],
                    pair3(pab[:], 256, 128), AF.Exp)
                nc.vector.tensor_copy(
                    pair3(bto_rot[:, r * WO:(r + 2) * WO], WO, 128),
                    pab[:].rearrange("p (two w) -> p two w", two=2)
                    [:, :, 128:256])
                if nk >= 2:
                    g_mm(nk - 2)
                    g_mm(nk - 1)
            # V projection over this half's 4 512-tiles
            for q in range(4):
                nt = h * 4 + q
                pv = ps_w.tile([128, 512], F32, name=f"pv{s}_{nt}",
                               tag="ps_w")
                for c in range(CC):
                    nc.tensor.matmul(
                        pv[:], wvt[c], xh[c][:, q * 512:(q + 1) * 512],
                        start=(c == 0), stop=(c == CC - 1))
                nc.scalar.activation(
                    expv[:, nt * 512:(nt + 1) * 512], pv[:], AF.Exp)
        g_mm(NK - 2)
        g_mm(NK - 1)
        rsa_inv = sp.tile([128, 1], F32, name=f"rsa{s}", tag="small")
        nc.vector.reciprocal(rsa_inv[:], pg[:, 128:129])
        nc.scalar.mul(gt[:, 0:128], pg[:, 0:128], rsa_inv[:])

        # Y^T chunk-PAIRS (+ col sums in col 128 of each 130-wide half),
        # scaled per-partition by 1/colsum(expV)
        pm = ps_s.tile([128, WO], F32, name=f"pm{s}", tag="ps_s")

        def m_mm(j):
            nc.tensor.matmul(pm[:], yt_c(s, j, full=False), yt_c(s, j),
                             start=(j == 0), stop=(j == NK - 1))
        for jp in range(16):
            nk = jp * 2
            py = ps_y.tile([128, 2 * WO], F32, name=f"py{s}_{nk}",
                           tag="ps_y")
            for u in range(2):
                nc.tensor.matmul(
                    py[:, u * WO:(u + 1) * WO],
                    expv[:, (nk + u) * 128:(nk + u + 1) * 128], gt[:])
            sinv2 = sp.tile([128, 2], F32, name=f"sinv{s}_{nk}", tag="sm2",
                            bufs=4)
            nc.vector.reciprocal(sinv2[:], py[:, 128::WO])
            if jp % 2 == 0:
                # both chunks in one broadcast tensor_tensor on vector
                nc.vector.tensor_tensor(
                    pair3(yt_bufs[s][:, nk * WO:(nk + 2) * WO], WO, 128),
                    pair3(py[:], WO, 128),
                    sinv2[:].unsqueeze(2).to_broadcast((128, 2, 128)),
                    ALU.mult)
            else:
                for u in range(2):
                    nc.scalar.mul(yt_c(s, nk + u, full=False),
                                  py[:, u * WO:u * WO + 128],
                                  sinv2[:, u:u + 1])
            if nk >= 2:
                m_mm(nk - 2)
                m_mm(nk - 1)
        m_mm(NK - 2)
        m_mm(NK - 1)
        if s == 0:
            nc.vector.tensor_copy(macc[:], pm[:])
        else:
            nc.vector.tensor_add(macc[:], macc[:], pm[:])

        # Y stored bf16 for Z.  Sample 0's transposes run right here --
        # they fill the PE idle gap while sample 1's x is still loading;
        # sample 1's go under the AllReduce.
        y = y_p.tile([128, N], BF16, name=f"y{s}", tag="y")
        ys.append(y)
        if s == 0:
            for nk in range(NK):
                ptr = ps_y.tile([128, 128], BF16, name=f"ptr0_{nk}",
                                tag="ps_y")
                nc.tensor.transpose(ptr[:], yt_c(0, nk, full=False),
                                    ident_b[:])
                if nk % 2 == 0:
                    nc.scalar.copy(y[:, nk * 128:(nk + 1) * 128], ptr[:])
                else:
                    nc.vector.tensor_copy(y[:, nk * 128:(nk + 1) * 128],
                                          ptr[:])

    # ---------------- local Z-space stats + 4KB f32 AllReduce -------------
    # one packed PSUM bank: pt_all[:, c*WO:(c+1)*WO] = wP_c @ [M | sumY]
    # stats_l[:, c]    = sumZ chunk c   (col 128 of each WO block)
    # stats_l[:, CC+c] = sumZ2 chunk c  = diag(wP M wP^T) chunk
    mg_b = cp.tile([128, WO], BF16, name="mg_b")
    nc.vector.tensor_copy(mg_b[:], macc[:])
    stats_l = cp.tile([128, 2 * CC], F32, name="stats_l")
    prod = sp.tile([128, C], F32, name="prod", tag="prod", bufs=1)
    for cp2 in range(2):  # c-chunk pairs (2*WO fits one PSUM bank)
        pt2 = ps_y.tile([128, 2 * WO], F32, name=f"pt2_{cp2}", tag="ps_y")
        for u in range(2):
            c = cp2 * 2 + u
            nc.tensor.matmul(pt2[:, u * WO:(u + 1) * WO],
                             wpt_b[:, c * 128:(c + 1) * 128], mg_b[:])
        nc.scalar.copy(stats_l[:, cp2 * 2:cp2 * 2 + 2], pt2[:, 128::WO])
        nc.vector.tensor_tensor(
            prod[:, cp2 * 256:(cp2 + 1) * 256]
            .rearrange("p (cc w) -> p cc w", cc=2),
            pair3(pt2[:], WO, 128),
            wpn_all[:, cp2 * 256:(cp2 + 1) * 256]
            .rearrange("p (cc w) -> p cc w", cc=2),
            ALU.mult)
        nc.vector.tensor_reduce(
            stats_l[:, CC + cp2 * 2:CC + cp2 * 2 + 2].unsqueeze(2),
            prod[:, cp2 * 256:(cp2 + 1) * 256]
            .rearrange("p (cc w) -> p cc w", cc=2),
            axis=mybir.AxisListType.X, op=ALU.add)

    ar_in = dp.tile([128, 2 * CC], F32, name="ar_in")
    ar_out = dp.tile([128, 2 * CC], F32, name="ar_out", addr_space="Shared")
    nc.scalar.dma_start(ar_in[:], stats_l[:])
    nc.gpsimd.collective_compute(
        "AllReduce", ALU.add,
        replica_groups=[list(range(N_CORES))],
        ins=[ar_in.opt()], outs=[ar_out.opt()])
    stats_g = cp.tile([128, 2 * CC], F32, name="stats_g")
    nc.sync.dma_start(stats_g[:], ar_out[:])

    # sample 1's transposes and both samples' Z run under the AllReduce
    for nk in range(NK):
        ptr = ps_y.tile([128, 128], BF16, name=f"ptr1_{nk}", tag="ps_y")
        nc.tensor.transpose(ptr[:], yt_c(1, nk, full=False), ident_b[:])
        if nk % 2 == 0:
            nc.scalar.copy(ys[1][:, nk * 128:(nk + 1) * 128], ptr[:])
        else:
            nc.vector.tensor_copy(ys[1][:, nk * 128:(nk + 1) * 128],
                                  ptr[:])
    zbs = [[], []]
    zu = 0
    for zs in range(B_LOCAL):
        for c in range(CC):
            zt = zb_p.tile([128, N], BF16, name=f"zb{zs}_{c}", tag="zb")
            zbs[zs].append(zt)
            for nt in range(8):
                pz = ps_w.tile([128, 512], F32, name=f"pz{zs}_{c}_{nt}",
                               tag="ps_w")
                nc.tensor.matmul(
                    pz[:], wpt_b[:, c * 128:(c + 1) * 128],
                    ys[zs][:, nt * 512:(nt + 1) * 512])
                if zu % 2 == 0:
                    nc.scalar.copy(zt[:, nt * 512:(nt + 1) * 512], pz[:])
                else:
                    nc.vector.tensor_copy(zt[:, nt * 512:(nt + 1) * 512],
                                          pz[:])
                zu += 1

    # ---------------- BN affine coefficients (post-AR, tiny) --------------
    # kept on vector (+ one scalar Sqrt) to minimize cross-engine hops
    mz = cp.tile([128, CC], F32, name="mz")
    ezz = cp.tile([128, CC], F32, name="ezz")
    a_all = cp.tile([128, CC], F32, name="a_all")
    b_all = cp.tile([128, CC], F32, name="b_all")
    nc.vector.tensor_scalar_mul(mz[:], stats_g[:, 0:CC], 1.0 / BN_COUNT)
    nc.vector.tensor_scalar_mul(ezz[:], stats_g[:, CC:2 * CC], 1.0 / BN_COUNT)
    sq = sp.tile([128, CC], F32, name="sq", tag="small4")
    nc.vector.tensor_tensor(sq[:], mz[:], mz[:], ALU.mult)
    var = sp.tile([128, CC], F32, name="var", tag="small4")
    nc.vector.tensor_tensor(var[:], ezz[:], sq[:], ALU.subtract)
    std = sp.tile([128, CC], F32, name="std", tag="small4")
    nc.scalar.activation(std[:], var[:], AF.Sqrt, bias=eps_t[:])
    rstd = sp.tile([128, CC], F32, name="rstd", tag="small4")
    nc.vector.reciprocal(rstd[:], std[:])
    nc.vector.tensor_tensor(a_all[:], gamma_sb[:], rstd[:], ALU.mult)
    tmp_b = sp.tile([128, CC], F32, name="tmp_b", tag="small4")
    nc.vector.tensor_tensor(tmp_b[:], mz[:], a_all[:], ALU.mult)
    nc.vector.tensor_tensor(b_all[:], beta_sb[:], tmp_b[:], ALU.subtract)

    # ---------------- phase 3: affine+residual, bf16 out ------------------
    # 16 [128, 2048] units, 5-deep outb pipelining.  Affine: scalar
    # activation for 8, vector tensor_scalar for 8; residual adds on
    # vector except every 4th on gpsimd (slow per-op but it idles here);
    # stores round-robin the sync/scalar/gpsimd queues.
    unit = 0
    for s in range(B_LOCAL):
        for c in range(CC):
            cs = slice(c * 128, (c + 1) * 128)
            for h in range(2):
                hs = slice(h * HALF, (h + 1) * HALF)
                outb = ob_p.tile([128, HALF], BF16, name=f"outb{s}_{c}_{h}",
                                 tag="outb")
                if unit % 2 == 0:
                    nc.scalar.activation(
                        outb[:], zbs[s][c][:, hs], AF.Identity,
                        bias=b_all[:, c:c + 1], scale=a_all[:, c:c + 1])
                else:
                    nc.vector.tensor_scalar(
                        outb[:], zbs[s][c][:, hs], a_all[:, c:c + 1],
                        b_all[:, c:c + 1], ALU.mult, ALU.add)
                radd = nc.gpsimd if unit % 4 == 3 else nc.vector
                radd.tensor_tensor(outb[:], outb[:],
                                   x_res[s][h][c][:], ALU.add)
                eng = (nc.sync, nc.scalar, nc.gpsimd)[unit % 3]
                eng.dma_start(out_ext.ap()[s, cs, hs], outb[:])
                unit += 1


def build_graph():
    nc = bacc.Bacc("TRN2", target_bir_lowering=False, num_devices=N_CORES)

    x_ext = nc.dram_tensor("x", (B_LOCAL, C, N), F32, kind="ExternalInput")
    wabt_ext = nc.dram_tensor("wabt", (128, CC * 256), BF16,
                              kind="ExternalInput")
    wvt_ext = nc.dram_tensor("wvt", (128, CC * 128), BF16,
                             kind="ExternalInput")
    wptb_ext = nc.dram_tensor("wptb", (128, C), BF16, kind="ExternalInput")
    wpn_ext = nc.dram_tensor("wpn", (128, C), F32, kind="ExternalInput")
    gbt_ext = nc.dram_tensor("gbt", (128, 2 * CC), F32, kind="ExternalInput")
    idb_ext = nc.dram_tensor("identb", (128, 128), BF16,
                             kind="ExternalInput")
    out_ext = nc.dram_tensor("out", (B_LOCAL, C, N), BF16,
                             kind="ExternalOutput")
    ext = (x_ext, wabt_ext, wvt_ext, wptb_ext, wpn_ext, gbt_ext, idb_ext,
           out_ext)

    with tile.TileContext(nc) as tc:
        with (
            tc.tile_pool(name="const", bufs=1) as cp,
            tc.tile_pool(name="xp", bufs=16) as xp,
            tc.tile_pool(name="expv", bufs=1) as expv_p,
            tc.tile_pool(name="ybig", bufs=B_LOCAL) as y_p,
            tc.tile_pool(name="zb", bufs=8) as zb_p,
            tc.tile_pool(name="small", bufs=8) as sp,
            tc.tile_pool(name="outb", bufs=5) as ob_p,
            tc.tile_pool(name="ps_ab", bufs=2, space="PSUM") as ps_ab,
            tc.tile_pool(name="ps_w", bufs=2, space="PSUM") as ps_w,
            tc.tile_pool(name="ps_y", bufs=2, space="PSUM") as ps_y,
            tc.tile_pool(name="ps_s", bufs=2, space="PSUM") as ps_s,
            tc.tile_pool(name="dram", bufs=1, space="DRAM") as dp,
        ):
            _body(nc, cp, xp, expv_p, y_p, sp, ob_p, zb_p,
                  ps_ab, ps_w, ps_y, ps_s, dp, ext)

    nc.compile()
    return nc


_NC = None


def _get_nc():
    global _NC
    if _NC is None:
        _NC = build_graph()
    return _NC


def _prep_weights(wA, wB, wV, wP, gamma, beta):
    """Host-side layout marshalling: transposed, bf16-cast weight blocks."""
    wA = np.asarray(wA, dtype=np.float32)
    wB = np.asarray(wB, dtype=np.float32)
    wV = np.asarray(wV, dtype=np.float32)
    wP = np.asarray(wP, dtype=np.float32)
    gamma = np.asarray(gamma, dtype=np.float32)
    beta = np.asarray(beta, dtype=np.float32)
    bf = ml_dtypes.bfloat16
    # wabt[p, c*256+j]: rows = channel-within-chunk, [wA^T | wB^T] blocks
    wat = wA.T.reshape(CC, 128, INTER)  # [c, p, i]
    wbt = wB.T.reshape(CC, 128, INTER)
    wabt = np.concatenate([wat, wbt], axis=2)  # [c, p, 256]
    wabt = np.ascontiguousarray(
        wabt.transpose(1, 0, 2).reshape(128, CC * 256)).astype(bf)
    wvt = np.ascontiguousarray(
        wV.T.reshape(CC, 128, INTER).transpose(1, 0, 2)
        .reshape(128, CC * 128)).astype(bf)
    # wptb[p=i, c]: wP^T
    wptb = np.ascontiguousarray(wP.T).astype(bf)
    # wpn[p, c*128+j] = wP[c*128+p, j] per 128-row chunk, side by side
    wpn = np.ascontiguousarray(
        wP.reshape(CC, 128, INTER).transpose(1, 0, 2)
        .reshape(128, CC * 128)).astype(np.float32)
    gbt = np.ascontiguousarray(
        np.concatenate([gamma.reshape(CC, 128).T,
                        beta.reshape(CC, 128).T], axis=1)).astype(np.float32)
    identb = np.eye(128, dtype=np.float32).astype(bf)
    return {"wabt": wabt, "wvt": wvt, "wptb": wptb, "wpn": wpn, "gbt": gbt,
            "identb": identb}


def kernel(x, wA, wB, wV, wP, gamma, beta):
    x = np.ascontiguousarray(np.asarray(x, dtype=np.float32))
    shards = x.reshape(N_CORES, B_LOCAL, C, N)
    common = _prep_weights(wA, wB, wV, wP, gamma, beta)
    in_maps = [dict(common, x=np.ascontiguousarray(shards[i]))
               for i in range(N_CORES)]
    nc = _get_nc()
    res = run_bass_kernel_spmd(nc, in_maps, core_ids=list(range(N_CORES)))
    out = np.concatenate(
        [np.asarray(res.results[i]["out"]).astype(np.float32)
         for i in range(N_CORES)], axis=0)
    return out.reshape(B_GLOBAL, C, H, W)


# revision 33
# speedup vs baseline: 1.0045x; 1.0045x over previous
"""Trainium2 Bass kernel for sparse_attention nn_A2_42752104464871.

Reference computation (per batch sample b):
    xr = x[b] reshaped (C=512, N=4096)
    A = wA @ xr; B = wB @ xr; V = wV @ xr          (INTER=128 each)
    A_attn = softmax(A, axis=N); V_attn = softmax(V, axis=inter)
    G = B @ A_attn^T ; Y = G @ V_attn ; Z = wP @ Y
    out = x + BN(Z) * gamma + beta                 (BN over batch+spatial)

Distribution: pure data-parallel, 2 samples per core on 8 NeuronCores;
weights replicated.  The only cross-core traffic is a 4 KB AllReduce of
the per-channel Z sums and squared sums (the wP projection of the
Y-space second moment is applied locally, before the reduce).

Implementation notes:
  - the kernel is tensor-engine-bound in phase 1 (the PE runs under a
    power throttle at ~half nominal clock), so everything movable is
    kept off the PE: weights arrive pre-transposed/pre-cast from the
    host (layout marshalling in kernel()), and the gpsimd software DMA
    queue carries only the casting x flood.
  - the attention chain runs in bf16 with fp32 PSUM accumulate.
  - softmaxes skip max-subtraction (inputs are ~N(0,1); exp is safe).
  - row-sums of expA / column-sums of expV come free as persistent
    ones-columns appended to the G and Y^T matmul moving operands.
  - AB projections, exp(A) and the B^T drain run on PAIRED spatial
    chunks (full 2KB PSUM banks); the Y^T loop runs paired too, with a
    two-column reciprocal and a broadcast tensor_tensor scale per pair.
  - BN stats: M = sum_n [Y|1]^T [Y|1] on PSUM -> local projection into
    one packed PSUM bank -> single-op strided drains -> 4KB f32
    AllReduce (Shared output).  A dummy warmup AllReduce issued during
    the x flood absorbs the collective first-call latency.
  - all activation tables pre-warm during the DMA flood (Exp last so
    it is still resident when phase 1 starts).
  - sample 0's transposes fill the PE gap while sample 1's x loads;
    sample 1's transposes and both samples' Z matmuls run under the
    AllReduce.  Output is written bf16 with the affine split between
    scalar (activation, 8) and vector (tensor_scalar, 8), residual adds
    on vector with every 4th on gpsimd, and full-row stores
    round-robining the sync/scalar/gpsimd DMA queues.
"""

import numpy as np
import ml_dtypes

from concourse import bacc, mybir, tile
from concourse.bass_utils import run_bass_kernel_spmd

N_CORES = 8
B_GLOBAL = 16
B_LOCAL = B_GLOBAL // N_CORES  # 2
C = 512
CC = C // 128  # 4 chunks of channels
H = W = 64
N = H * W  # 4096
NK = N // 128  # 32 spatial chunks
HALF = N // 2  # 2048
INTER = 128
EPS = 1e-5
BN_COUNT = float(B_GLOBAL * N)  # 65536
WO = 130  # chunk width incl. the two ones-columns
ROT = 4  # rotating slots for expa/bto chunk buffers (must be even)

F32 = mybir.dt.float32
BF16 = mybir.dt.bfloat16
AF = mybir.ActivationFunctionType
ALU = mybir.AluOpType


def _body(nc, cp, xp, expv_p, y_p, sp, ob_p, zb_p,
          ps_ab, ps_w, ps_y, ps_s, dp, ext):
    (x_ext, wabt_ext, wvt_ext, wptb_ext, wpn_ext, gbt_ext, idb_ext,
     out_ext) = ext

    # ---------------- preformatted weights on the scalar HW queue ---------
    # identity first: the PE warmup needs it earliest
    ident_b = cp.tile([128, 128], BF16, name="ident_b")
    wabt_all = cp.tile([128, CC * 256], BF16, name="wabt_all")
    wvt_all = cp.tile([128, CC * 128], BF16, name="wvt_all")
    wpt_b = cp.tile([128, C], BF16, name="wpt_b")
    wpn_all = cp.tile([128, C], F32, name="wpn_all")
    gb_sb = cp.tile([128, 2 * CC], F32, name="gb_sb")
    nc.scalar.dma_start(ident_b[:], idb_ext.ap())
    nc.scalar.dma_start(wabt_all[:], wabt_ext.ap())
    nc.scalar.dma_start(wvt_all[:], wvt_ext.ap())
    nc.scalar.dma_start(wpt_b[:], wptb_ext.ap())
    nc.scalar.dma_start(wpn_all[:], wpn_ext.ap())
    nc.scalar.dma_start(gb_sb[:], gbt_ext.ap())
    wabt = [wabt_all[:, c * 256:(c + 1) * 256] for c in range(CC)]
    wvt = [wvt_all[:, c * 128:(c + 1) * 128] for c in range(CC)]
    gamma_sb = gb_sb[:, 0:CC]
    beta_sb = gb_sb[:, CC:2 * CC]

    # warmup AllReduce triggered BEFORE the flood: the input is an
    # inline (NEFF-embedded) DRAM constant so the gpsimd doorbell has
    # NOTHING to wait on -- it fires ~10.5us in on every core with
    # minimal skew, completes under the x flood, and pays the collective
    # first-call latency well clear of the real AllReduce.  (A doorbell
    # that waits mid-queue stalls the software-DGE x transfers too.)
    eps_t = cp.tile([128, 1], F32, name="eps_t")
    nc.vector.memset(eps_t[:], EPS)
    war_in = nc.inline_tensor(np.zeros((128, 1), dtype=np.float32),
                              name="war_in")
    war_out = dp.tile([128, 1], F32, name="war_out", addr_space="Shared")
    nc.gpsimd.collective_compute(
        "AllReduce", ALU.add,
        replica_groups=[list(range(N_CORES))],
        ins=[war_in.ap()], outs=[war_out.opt()])

    # x flood (gpsimd: only it can cast f32->bf16).  Sample 0 lands at
    # quarter granularity (fast pipeline fill + no PE feed gaps); sample
    # 1 uses half-granularity DMAs (fewer gpsimd dispatch stalls).
    x_res = []
    for s in range(B_LOCAL):
        xhs = []
        for h in range(2):
            xh = [xp.tile([128, HALF], BF16, name=f"x{s}h{h}c{c}", tag="x")
                  for c in range(CC)]
            if s == 0:
                for q in range(4):
                    for c in range(CC):
                        off = h * HALF + q * 512
                        nc.gpsimd.dma_start(
                            xh[c][:, q * 512:(q + 1) * 512],
                            x_ext.ap()[s, c * 128:(c + 1) * 128,
                                       off:off + 512])
            else:
                for q in range(2):
                    for c in range(CC):
                        off = h * HALF + q * 1024
                        nc.gpsimd.dma_start(
                            xh[c][:, q * 1024:(q + 1) * 1024],
                            x_ext.ap()[s, c * 128:(c + 1) * 128,
                                       off:off + 1024])
            xhs.append(xh)
        x_res.append(xhs)

    # rotating spatial-chunk buffers; persistent yt (all 32 chunks live).
    # only the ones-COLUMNS of yt need presetting (cols 128:130 of each
    # 130-wide chunk) -- a strided memset, not a 1MB fill.
    expa_rot = cp.tile([128, ROT * 128], BF16, name="expa_rot")
    bto_rot = cp.tile([128, ROT * WO], BF16, name="bto_rot")
    yt_bufs = [cp.tile([128, NK * WO], BF16, name=f"yt_big{s}")
               for s in range(B_LOCAL)]
    for r in range(ROT):
        nc.vector.memset(bto_rot[:, r * WO + 128:r * WO + WO], 1.0)
    for t in yt_bufs:
        nc.vector.memset(
            t[:].rearrange("p (nk w) -> p nk w", nk=NK)[:, :, 128:130], 1.0)
    gt = cp.tile([128, WO], BF16, name="gt")
    nc.vector.memset(gt[:, 128:130], 1.0)

    # pre-warm every activation table used later so no ~1.3us
    # ACT_TABLE_LOAD lands on the critical path (Exp LAST so it is the
    # resident table when the first expa drain issues)
    warm = sp.tile([128, 1], F32, name="warm", tag="small")
    nc.scalar.activation(warm[:], eps_t[:], AF.Identity)
    nc.scalar.activation(warm[:], eps_t[:], AF.Square)
    nc.scalar.activation(warm[:], eps_t[:], AF.Sqrt)
    nc.scalar.activation(warm[:], eps_t[:], AF.Exp)
    warm_v = sp.tile([128, 1], F32, name="warm_v", tag="small")
    nc.vector.reciprocal(warm_v[:], eps_t[:])

    # short PE warmup while the x DMAs stream
    pwu = ps_w.tile([128, 128], F32, name="pwu", tag="ps_w")
    for i in range(4):
        nc.tensor.matmul(pwu[:], ident_b[:], ident_b[:],
                         start=(i == 0), stop=(i == 3))
    wu_sink = sp.tile([128, 1], F32, name="wu_sink", tag="small")
    nc.vector.tensor_copy(wu_sink[:], pwu[:, 0:1])

    macc = cp.tile([128, WO], F32, name="macc")

    def expa_c(nk):
        r = nk % ROT
        return expa_rot[:, r * 128:(r + 1) * 128]

    def bto_c(nk):
        r = nk % ROT
        return bto_rot[:, r * WO:r * WO + WO]

    def yt_c(s, nk, full=True):
        return yt_bufs[s][:, nk * WO:nk * WO + (WO if full else 128)]

    def pair3(ap2d, width, inner):
        """view a [128, 2*width] slice as [128, 2, inner] (inner<=width)"""
        return ap2d.rearrange("p (two w) -> p two w", two=2)[:, :, 0:inner]

    # ---------------- phase 1: per-sample ----------------
    ys = []
    for s in range(B_LOCAL):
        expv = expv_p.tile([128, N], BF16, name=f"expv{s}", tag="expv")
        pg = ps_s.tile([128, WO], F32, name=f"pg{s}", tag="ps_s")

        def g_mm(j):
            nc.tensor.matmul(pg[:], expa_c(j), bto_c(j),
                             start=(j == 0), stop=(j == NK - 1))
        for h in range(2):
            xh = x_res[s][h]
            # A^T/B^T joint projection over this half's 8 chunk-PAIRS
            for kp in range(8):
                nk = h * 16 + kp * 2  # first chunk of the pair (even)
                pab = ps_ab.tile([128, 512], F32, name=f"pab{s}_{nk}",
                                 tag="ps_ab")
                for u in range(2):
                    k = kp * 2 + u
                    for c in range(CC):
                        nc.tensor.matmul(
                            pab[:, u * 256:(u + 1) * 256],
                            xh[c][:, k * 128:(k + 1) * 128], wabt[c],
                            start=(c == 0), stop=(c == CC - 1))
                # paired drains: exp(A) on scalar, B^T cast-copy on vector
                r = nk % ROT
                nc.scalar.activation(
                    expa_rot[:, r * 128:(r + 2) * 128],
                    pair3(pab[:], 256, 128), AF.Exp)
                nc.vector.tensor_copy(
                    pair3(bto_rot[:, r * WO:(r + 2) * WO], WO, 128),
                    pab[:].rearrange("p (two w) -> p two w", two=2)
                    [:, :, 128:256])
                if nk >= 2:
                    g_mm(nk - 2)
                    g_mm(nk - 1)
            # V projection over this half's 4 512-tiles
            for q in range(4):
                nt = h * 4 + q
                pv = ps_w.tile([128, 512], F32, name=f"pv{s}_{nt}",
                               tag="ps_w")
                for c in range(CC):
                    nc.tensor.matmul(
                        pv[:], wvt[c], xh[c][:, q * 512:(q + 1) * 512],
                        start=(c == 0), stop=(c == CC - 1))
                nc.scalar.activation(
                    expv[:, nt * 512:(nt + 1) * 512], pv[:], AF.Exp)
        g_mm(NK - 2)
        g_mm(NK - 1)
        rsa_inv = sp.tile([128, 1], F32, name=f"rsa{s}", tag="small")
        nc.vector.reciprocal(rsa_inv[:], pg[:, 128:129])
        nc.scalar.mul(gt[:, 0:128], pg[:, 0:128], rsa_inv[:])

        # Y^T chunk-PAIRS (+ col sums in col 128 of each 130-wide half),
        # scaled per-partition by 1/colsum(expV)
        pm = ps_s.tile([128, WO], F32, name=f"pm{s}", tag="ps_s")

        def m_mm(j):
            nc.tensor.matmul(pm[:], yt_c(s, j, full=False), yt_c(s, j),
                             start=(j == 0), stop=(j == NK - 1))
        for jp in range(16):
            nk = jp * 2
            py = ps_y.tile([128, 2 * WO], F32, name=f"py{s}_{nk}",
                           tag="ps_y")
            for u in range(2):
                nc.tensor.matmul(
                    py[:, u * WO:(u + 1) * WO],
                    expv[:, (nk + u) * 128:(nk + u + 1) * 128], gt[:])
            sinv2 = sp.tile([128, 2], F32, name=f"sinv{s}_{nk}", tag="sm2",
                            bufs=4)
            nc.vector.reciprocal(sinv2[:], py[:, 128::WO])
            if jp % 2 == 0:
                # both chunks in one broadcast tensor_tensor on vector
                nc.vector.tensor_tensor(
                    pair3(yt_bufs[s][:, nk * WO:(nk + 2) * WO], WO, 128),
                    pair3(py[:], WO, 128),
                    sinv2[:].unsqueeze(2).to_broadcast((128, 2, 128)),
                    ALU.mult)
            else:
                for u in range(2):
                    nc.scalar.mul(yt_c(s, nk + u, full=False),
                                  py[:, u * WO:u * WO + 128],
                                  sinv2[:, u:u + 1])
            if nk >= 2:
                m_mm(nk - 2)
                m_mm(nk - 1)
        m_mm(NK - 2)
        m_mm(NK - 1)
        if s == 0:
            nc.vector.tensor_copy(macc[:], pm[:])
        else:
            nc.vector.tensor_add(macc[:], macc[:], pm[:])

        # Y stored bf16 for Z.  Sample 0's transposes run right here --
        # they fill the PE idle gap while sample 1's x is still loading;
        # sample 1's go under the AllReduce.
        y = y_p.tile([128, N], BF16, name=f"y{s}", tag="y")
        ys.append(y)
        if s == 0:
            for nk in range(NK):
                ptr = ps_y.tile([128, 128], BF16, name=f"ptr0_{nk}",
                                tag="ps_y")
                nc.tensor.transpose(ptr[:], yt_c(0, nk, full=False),
                                    ident_b[:])
                if nk % 2 == 0:
                    nc.scalar.copy(y[:, nk * 128:(nk + 1) * 128], ptr[:])
                else:
                    nc.vector.tensor_copy(y[:, nk * 128:(nk + 1) * 128],
                                          ptr[:])

    # ---------------- local Z-space stats + 4KB f32 AllReduce -------------
    # one packed PSUM bank: pt_all[:, c*WO:(c+1)*WO] = wP_c @ [M | sumY]
    # stats_l[:, c]    = sumZ chunk c   (col 128 of each WO block)
    # stats_l[:, CC+c] = sumZ2 chunk c  = diag(wP M wP^T) chunk
    mg_b = cp.tile([128, WO], BF16, name="mg_b")
    nc.vector.tensor_copy(mg_b[:], macc[:])
    stats_l = cp.tile([128, 2 * CC], F32, name="stats_l")
    prod = sp.tile([128, C], F32, name="prod", tag="prod", bufs=1)
    for cp2 in range(2):  # c-chunk pairs (2*WO fits one PSUM bank)
        pt2 = ps_y.tile([128, 2 * WO], F32, name=f"pt2_{cp2}", tag="ps_y")
        for u in range(2):
            c = cp2 * 2 + u
            nc.tensor.matmul(pt2[:, u * WO:(u + 1) * WO],
                             wpt_b[:, c * 128:(c + 1) * 128], mg_b[:])
        nc.scalar.copy(stats_l[:, cp2 * 2:cp2 * 2 + 2], pt2[:, 128::WO])
        nc.vector.tensor_tensor(
            prod[:, cp2 * 256:(cp2 + 1) * 256]
            .rearrange("p (cc w) -> p cc w", cc=2),
            pair3(pt2[:], WO, 128),
            wpn_all[:, cp2 * 256:(cp2 + 1) * 256]
            .rearrange("p (cc w) -> p cc w", cc=2),
            ALU.mult)
        nc.vector.tensor_reduce(
            stats_l[:, CC + cp2 * 2:CC + cp2 * 2 + 2].unsqueeze(2),
            prod[:, cp2 * 256:(cp2 + 1) * 256]
            .rearrange("p (cc w) -> p cc w", cc=2),
            axis=mybir.AxisListType.X, op=ALU.add)

    ar_in = dp.tile([128, 2 * CC], F32, name="ar_in")
    ar_out = dp.tile([128, 2 * CC], F32, name="ar_out", addr_space="Shared")
    nc.scalar.dma_start(ar_in[:], stats_l[:])
    nc.gpsimd.collective_compute(
        "AllReduce", ALU.add,
        replica_groups=[list(range(N_CORES))],
        ins=[ar_in.opt()], outs=[ar_out.opt()])
    stats_g = cp.tile([128, 2 * CC], F32, name="stats_g")
    nc.sync.dma_start(stats_g[:], ar_out[:])

    # sample 1's transposes and both samples' Z run under the AllReduce
    for nk in range(NK):
        ptr = ps_y.tile([128, 128], BF16, name=f"ptr1_{nk}", tag="ps_y")
        nc.tensor.transpose(ptr[:], yt_c(1, nk, full=False), ident_b[:])
        if nk % 2 == 0:
            nc.scalar.copy(ys[1][:, nk * 128:(nk + 1) * 128], ptr[:])
        else:
            nc.vector.tensor_copy(ys[1][:, nk * 128:(nk + 1) * 128],
                                  ptr[:])
    zbs = [[], []]
    zu = 0
    for zs in range(B_LOCAL):
        for c in range(CC):
            zt = zb_p.tile([128, N], BF16, name=f"zb{zs}_{c}", tag="zb")
            zbs[zs].append(zt)
            for nt in range(8):
                pz = ps_w.tile([128, 512], F32, name=f"pz{zs}_{c}_{nt}",
                               tag="ps_w")
                nc.tensor.matmul(
                    pz[:], wpt_b[:, c * 128:(c + 1) * 128],
                    ys[zs][:, nt * 512:(nt + 1) * 512])
                if zu % 2 == 0:
                    nc.scalar.copy(zt[:, nt * 512:(nt + 1) * 512], pz[:])
                else:
                    nc.vector.tensor_copy(zt[:, nt * 512:(nt + 1) * 512],
                                          pz[:])
                zu += 1

    # ---------------- BN affine coefficients (post-AR, tiny) --------------
    # kept on vector (+ one scalar Sqrt) to minimize cross-engine hops
    mz = cp.tile([128, CC], F32, name="mz")
    ezz = cp.tile([128, CC], F32, name="ezz")
    a_all = cp.tile([128, CC], F32, name="a_all")
    b_all = cp.tile([128, CC], F32, name="b_all")
    nc.vector.tensor_scalar_mul(mz[:], stats_g[:, 0:CC], 1.0 / BN_COUNT)
    nc.vector.tensor_scalar_mul(ezz[:], stats_g[:, CC:2 * CC], 1.0 / BN_COUNT)
    sq = sp.tile([128, CC], F32, name="sq", tag="small4")
    nc.vector.tensor_tensor(sq[:], mz[:], mz[:], ALU.mult)
    var = sp.tile([128, CC], F32, name="var", tag="small4")
    nc.vector.tensor_tensor(var[:], ezz[:], sq[:], ALU.subtract)
    std = sp.tile([128, CC], F32, name="std", tag="small4")
    nc.scalar.activation(std[:], var[:], AF.Sqrt, bias=eps_t[:])
    rstd = sp.tile([128, CC], F32, name="rstd", tag="small4")
    nc.vector.reciprocal(rstd[:], std[:])
    nc.vector.tensor_tensor(a_all[:], gamma_sb[:], rstd[:], ALU.mult)
    tmp_b = sp.tile([128, CC], F32, name="tmp_b", tag="small4")
    nc.vector.tensor_tensor(tmp_b[:], mz[:], a_all[:], ALU.mult)
    nc.vector.tensor_tensor(b_all[:], beta_sb[:], tmp_b[:], ALU.subtract)

    # ---------------- phase 3: affine+residual, bf16 out ------------------
    # 16 [128, 2048] units, 5-deep outb pipelining.  Affine: scalar
    # activation for 8, vector tensor_scalar for 8; residual adds on
    # vector except every 4th on gpsimd (slow per-op but it idles here);
    # stores round-robin the sync/scalar/gpsimd queues.
    unit = 0
    for s in range(B_LOCAL):
        for c in range(CC):
            cs = slice(c * 128, (c + 1) * 128)
            for h in range(2):
                hs = slice(h * HALF, (h + 1) * HALF)
                outb = ob_p.tile([128, HALF], BF16, name=f"outb{s}_{c}_{h}",
                                 tag="outb")
                if unit % 2 == 0:
                    nc.scalar.activation(
                        outb[:], zbs[s][c][:, hs], AF.Identity,
                        bias=b_all[:, c:c + 1], scale=a_all[:, c:c + 1])
                else:
                    nc.vector.tensor_scalar(
                        outb[:], zbs[s][c][:, hs], a_all[:, c:c + 1],
                        b_all[:, c:c + 1], ALU.mult, ALU.add)
                radd = nc.gpsimd if unit % 4 == 3 else nc.vector
                radd.tensor_tensor(outb[:], outb[:],
                                   x_res[s][h][c][:], ALU.add)
                eng = (nc.sync, nc.scalar, nc.gpsimd)[unit % 3]
                eng.dma_start(out_ext.ap()[s, cs, hs], outb[:])
                unit += 1


def build_graph():
    nc = bacc.Bacc("TRN2", target_bir_lowering=False, num_devices=N_CORES)

    x_ext = nc.dram_tensor("x", (B_LOCAL, C, N), F32, kind="ExternalInput")
    wabt_ext = nc.dram_tensor("wabt", (128, CC * 256), BF16,
                              kind="ExternalInput")
    wvt_ext = nc.dram_tensor("wvt", (128, CC * 128), BF16,
                             kind="ExternalInput")
    wptb_ext = nc.dram_tensor("wptb", (128, C), BF16, kind="ExternalInput")
    wpn_ext = nc.dram_tensor("wpn", (128, C), F32, kind="ExternalInput")
    gbt_ext = nc.dram_tensor("gbt", (128, 2 * CC), F32, kind="ExternalInput")
    idb_ext = nc.dram_tensor("identb", (128, 128), BF16,
                             kind="ExternalInput")
    out_ext = nc.dram_tensor("out", (B_LOCAL, C, N), BF16,
                             kind="ExternalOutput")
    ext = (x_ext, wabt_ext, wvt_ext, wptb_ext, wpn_ext, gbt_ext, idb_ext,
           out_ext)

    with tile.TileContext(nc) as tc:
        with (
            tc.tile_pool(name="const", bufs=1) as cp,
            tc.tile_pool(name="xp", bufs=16) as xp,
            tc.tile_pool(name="expv", bufs=1) as expv_p,
            tc.tile_pool(name="ybig", bufs=B_LOCAL) as y_p,
            tc.tile_pool(name="zb", bufs=8) as zb_p,
            tc.tile_pool(name="small", bufs=8) as sp,
            tc.tile_pool(name="outb", bufs=5) as ob_p,
            tc.tile_pool(name="ps_ab", bufs=2, space="PSUM") as ps_ab,
            tc.tile_pool(name="ps_w", bufs=2, space="PSUM") as ps_w,
            tc.tile_pool(name="ps_y", bufs=2, space="PSUM") as ps_y,
            tc.tile_pool(name="ps_s", bufs=2, space="PSUM") as ps_s,
            tc.tile_pool(name="dram", bufs=1, space="DRAM") as dp,
        ):
            _body(nc, cp, xp, expv_p, y_p, sp, ob_p, zb_p,
                  ps_ab, ps_w, ps_y, ps_s, dp, ext)

    nc.compile()
    return nc


_NC = None


def _get_nc():
    global _NC
    if _NC is None:
        _NC = build_graph()
    return _NC


def _prep_weights(wA, wB, wV, wP, gamma, beta):
    """Host-side layout marshalling: transposed, bf16-cast weight blocks."""
    wA = np.asarray(wA, dtype=np.float32)
    wB = np.asarray(wB, dtype=np.float32)
    wV = np.asarray(wV, dtype=np.float32)
    wP = np.asarray(wP, dtype=np.float32)
    gamma = np.asarray(gamma, dtype=np.float32)
    beta = np.asarray(beta, dtype=np.float32)
    bf = ml_dtypes.bfloat16
    # wabt[p, c*256+j]: rows = channel-within-chunk, [wA^T | wB^T] blocks
    wat = wA.T.reshape(CC, 128, INTER)  # [c, p, i]
    wbt = wB.T.reshape(CC, 128, INTER)
    wabt = np.concatenate([wat, wbt], axis=2)  # [c, p, 256]
    wabt = np.ascontiguousarray(
        wabt.transpose(1, 0, 2).reshape(128, CC * 256)).astype(bf)
    wvt = np.ascontiguousarray(
        wV.T.reshape(CC, 128, INTER).transpose(1, 0, 2)
        .reshape(128, CC * 128)).astype(bf)
    # wptb[p=i, c]: wP^T
    wptb = np.ascontiguousarray(wP.T).astype(bf)
    # wpn[p, c*128+j] = wP[c*128+p, j] per 128-row chunk, side by side
    wpn = np.ascontiguousarray(
        wP.reshape(CC, 128, INTER).transpose(1, 0, 2)
        .reshape(128, CC * 128)).astype(np.float32)
    gbt = np.ascontiguousarray(
        np.concatenate([gamma.reshape(CC, 128).T,
                        beta.reshape(CC, 128).T], axis=1)).astype(np.float32)
    identb = np.eye(128, dtype=np.float32).astype(bf)
    return {"wabt": wabt, "wvt": wvt, "wptb": wptb, "wpn": wpn, "gbt": gbt,
            "identb": identb}


def kernel(x, wA, wB, wV, wP, gamma, beta):
    x = np.ascontiguousarray(np.asarray(x, dtype=np.float32))
    shards = x.reshape(N_CORES, B_LOCAL, C, N)
    common = _prep_weights(wA, wB, wV, wP, gamma, beta)
    in_maps = [dict(common, x=np.ascontiguousarray(shards[i]))
               for i in range(N_CORES)]
    nc = _get_nc()
    res = run_bass_kernel_spmd(nc, in_maps, core_ids=list(range(N_CORES)))
    out = np.concatenate(
        [np.asarray(res.results[i]["out"]).astype(np.float32)
         for i in range(N_CORES)], axis=0)
    return out.reshape(B_GLOBAL, C, H, W)
